# revision 58
# baseline (speedup 1.0000x reference)
"""Trainium2 Bass kernel for nn_ContactAttention (B=2, L=512, E=128).

Design (v3):
  - Batch-split: cores 0-3 run the transformer for batch 0, cores 4-7 for
    batch 1 (conv+BN front runs both batches everywhere since BatchNorm stats
    span the batch dim; a data-driven 0/1 mask selects the core's batch).
  - LC layer host-symmetrized into per-cell coefficients A (pairs emb[:,s])
    and B (pairs emb[:,r]) on the lower triangle, then packed per OUTPUT
    ROW/COLUMN pair k: strip_k = [B[:,k,0:k] | A'[:,k:512,k]] (A' has the
    B-diagonal folded in), exactly 512 wide. Every strip multiplies the
    single emb column k (per-partition scalar -> TensorScalar 4x DVE mode)
    and one ones-window matmul column-reduces it into psum row k%128.
    128 strips per core. Host unpacks rows into the lower triangle.
  - conv1 via host-side im2col over (tap, channel) -> one matmul per batch.
  - All small weights ride in 4 packed DMAs (f32 pack + 3 bf16 packs).
  - LayerNorm via constant P = I - 1/128 matmul (x-mean in one matmul),
    variance from AVG @ (x-mean)^2, rstd row via ln/exp, row-broadcast via
    ones matmul. Softmax denominators ride along the AV matmul as 64 ones
    columns in the stationary. K-bias dropped (cancels in softmax).
"""
import sys

sys.path.insert(0, "/opt/trn_rl_repo")

import contextlib

import numpy as np
import ml_dtypes

import concourse.bass as bass
import concourse.mybir as mybir
import concourse.tile as tile
from concourse import bacc
from concourse.bass_utils import run_bass_kernel_spmd
from concourse.masks import make_identity

# Pin all activation functions to the one table set containing ln+exp+relu
# (avoids ~1.3us table reloads between ln/exp switches).
import concourse.hw_specs as _hw_specs

_orig_gat = _hw_specs.get_activation_tables


def _gat_pinned(module_arch):
    t = _orig_gat(module_arch)
    out = {}
    for name, fns in t.items():
        if name == "natural_log_exp_and_others":
            out[name] = fns
        else:
            out[name] = set()
    return out


_hw_specs.get_activation_tables = _gat_pinned
try:
    import concourse.bacc as _bacc_mod
    if hasattr(_bacc_mod, "get_activation_tables"):
        _bacc_mod.get_activation_tables = _gat_pinned
except Exception:
    pass

BF = ml_dtypes.bfloat16
dt = mybir.dt
F32, BF16 = dt.float32, dt.bfloat16
AL = mybir.AluOpType
AF = mybir.ActivationFunctionType

B, L, D, E, H, HD, FF = 2, 512, 127, 128, 2, 64, 2048
NCORES = 8
LP = 528             # padded conv length (8 + 512 + 8)
EPS = 1e-5

# f32 pack column layout
PF_BN1G, PF_BN1B, PF_BN2G, PF_BN2B = 0, 1, 2, 3
PF_QB, PF_OBE = 4, 7
PF_LN1G, PF_LN1B, PF_LN2G, PF_LN2B = 10, 13, 16, 19
PF_FB2, PF_FB1 = 22, 25
PF_BM0, PF_BM1, PF_BMQ = 73, 74, 75
PF_W = 79

# bf16 pack0a (conv1 im2col + weights, 36 rows), pack0b (conv2 weights)
P0_SEQ, P0_W1C = 0, 1024
P0A_W = 1024 + D
P0B_W = 9 * D
# bf16 packA (attention)
PA_IWQ, PA_IWK, PA_IWV, PA_OWT, PA_RV = 0, 384, 768, 1152, 1920
PA_OBP, PA_FBP = 2432, 2816     # rows 0: PM@obe, PM@fb2 per layer [1, 3E]
PA_W = 3200
# bf16 packB (ffn)
PB_W1, PB_W2 = 0, 6144
PB_W = 12288

_cached = {}


def _build():
    nc = bacc.Bacc("TRN2", target_bir_lowering=False, debug=False,
                   num_devices=NCORES)

    def din(name, shape, d=F32):
        return nc.dram_tensor(name, shape, d, kind="ExternalInput")

    packf_d = din("packf", (128, PF_W))
    pack0a_d = din("pack0a", (36, P0A_W), BF16)
    pack0b_d = din("pack0b", (D, P0B_W), BF16)
    packA_d = din("packA", (128, PA_W), BF16)
    packB_d = din("packB", (128, PB_W), BF16)
    wt_d = din("wt", (128, 128 * L), BF16)   # per-core LC strips
    res_d = nc.dram_tensor("res", (128, L), F32, kind="ExternalOutput")
    emb_d = nc.dram_tensor("embdbg", (E, L), BF16, kind="ExternalOutput")

    with tile.TileContext(nc) as tc, contextlib.ExitStack() as ctx:
        const = ctx.enter_context(tc.tile_pool(name="const", bufs=1))
        sc = ctx.enter_context(tc.tile_pool(name="sc", bufs=2))
        sc1 = ctx.enter_context(tc.tile_pool(name="sc1", bufs=1))
        scs = ctx.enter_context(tc.tile_pool(name="scs", bufs=2))
        hpool = ctx.enter_context(tc.tile_pool(name="hp", bufs=2))
        vap = ctx.enter_context(tc.tile_pool(name="vap", bufs=2))
        # PSUM: pss(2) + psP(2x2) + pso(2) = 8 banks
        pss = ctx.enter_context(tc.tile_pool(name="pss", bufs=2, space="PSUM"))
        psP = ctx.enter_context(tc.tile_pool(name="psP", bufs=2, space="PSUM"))
        pso = ctx.enter_context(tc.tile_pool(name="pso", bufs=2, space="PSUM"))
        vapool = ctx.enter_context(tc.tile_pool(name="vap2", bufs=4))

        def ld(dram, p, f, d=F32, pool=const):
            nm = "ld_" + dram.name
            t = pool.tile([p, f], d, tag=nm, name=nm)
            nc.sync.dma_start(t[:], dram.ap())
            return t

        # ---------- packed weight DMAs (queue order = urgency) ----------
        pack0a = ld(pack0a_d, 36, P0A_W, BF16)
        packf = ld(packf_d, 128, PF_W)
        pack0b = ld(pack0b_d, D, P0B_W, BF16)
        packA = ld(packA_d, 128, PA_W, BF16)
        packB = ld(packB_d, 128, PB_W, BF16)

        # xcur: pos row DMA'd from packA row 0 (cross-partition move).
        # MUST be queued before the big wt DMAs.
        xcur = vap.tile([E, L], BF16, tag="xc", name="xc0")
        nc.sync.dma_start(xcur[D:E, :], packA[0:1, PA_RV:PA_RV + L])

        # ---------- transversal weights (16.8MB, 8 chunked DMAs) ----------
        wt_t = const.tile([128, 128 * L], BF16, tag="wt", name="wt")
        NW = 8
        wstep = 128 * L // NW
        for i in range(NW):
            nc.sync.dma_start(wt_t[:, i * wstep:(i + 1) * wstep],
                              wt_d.ap()[:, i * wstep:(i + 1) * wstep])

        bn1g_t = packf[0:D, PF_BN1G:PF_BN1G + 1]
        bn1b_t = packf[0:D, PF_BN1B:PF_BN1B + 1]
        bn2g_t = packf[0:D, PF_BN2G:PF_BN2G + 1]
        bn2b_t = packf[0:D, PF_BN2B:PF_BN2B + 1]
        qb_t = packf[:, PF_QB:PF_QB + 3]
        obe_t = packf[:, PF_OBE:PF_OBE + 3]
        ln1g_t = packf[:, PF_LN1G:PF_LN1G + 3]
        ln1b_t = packf[:, PF_LN1B:PF_LN1B + 3]
        ln2g_t = packf[:, PF_LN2G:PF_LN2G + 3]
        ln2b_t = packf[:, PF_LN2B:PF_LN2B + 3]
        fb1_t = packf[:, PF_FB1:PF_FB1 + 48]
        fb2_t = packf[:, PF_FB2:PF_FB2 + 3]
        bm0_t = packf[0:D, PF_BM0:PF_BM0 + 1]
        bm1_t = packf[0:D, PF_BM1:PF_BM1 + 1]
        seq36 = pack0a[:, P0_SEQ:P0_SEQ + 1024]
        w1c_t = pack0a[:, P0_W1C:P0_W1C + D]
        w2Tt_t = pack0b[:]
        iwTq_t = packA[:, PA_IWQ:PA_IWQ + 384]
        iwTk_t = packA[:, PA_IWK:PA_IWK + 384]
        iwTv_t = packA[:, PA_IWV:PA_IWV + 384]
        owT_t = packA[0:64, PA_OWT:PA_OWT + 768]
        w1Tf_t = packB[:, PB_W1:PB_W1 + 3 * FF]
        w2Tf_t = packB[:, PB_W2:PB_W2 + 3 * 16 * E]

        # ---------- constants ----------
        AVG = const.tile([128, 128], BF16)
        nc.vector.memset(AVG[:], 1.0 / 128)
        PM = const.tile([128, 128], BF16)       # I - AVG (in-place build)
        make_identity(nc, PM[:])
        nc.vector.tensor_scalar(out=PM[:], in0=PM[:], scalar1=-1.0 / 128,
                                scalar2=None, op0=AL.add)
        ONEROW = const.tile([1, L], BF16)
        nc.vector.memset(ONEROW[:], 1.0)
        ONESW = const.tile([128, 256], BF16)    # sliding ones-column window
        nc.vector.memset(ONESW[:], 0.0)
        nc.vector.memset(ONESW[:, 128:129], 1.0)
        EPSC = const.tile([128, 1], F32)
        nc.vector.memset(EPSC[:], EPS)

        # ---------- conv front-end (both batches; pair = [b0|b1]) ----------
        def bn_stats_relu(ypair, g_t, b_t, out_aps):
            # ypair: 2 psum tiles [D, 512] (one per batch)
            s_col = scs.tile([D, 2], F32, tag="bns")
            q_col = scs.tile([D, 2], F32, tag="bnq")
            scr = sc1.tile([D, 1024], BF16, tag="xr")
            for b in range(2):
                nc.vector.tensor_reduce(s_col[:, b:b + 1], ypair[b][:],
                                        mybir.AxisListType.X, AL.add)
                nc.scalar.activation(scr[:, b * 512:(b + 1) * 512],
                                     ypair[b][:], AF.Square,
                                     accum_out=q_col[:, b:b + 1])
            mean = scs.tile([D, 1], F32, tag="bnm")
            nc.vector.tensor_reduce(mean[:], s_col[:], mybir.AxisListType.X,
                                    AL.add)
            nc.vector.tensor_scalar(out=mean[:], in0=mean[:],
                                    scalar1=1.0 / (2 * L), scalar2=None,
                                    op0=AL.mult)
            ex2 = scs.tile([D, 1], F32, tag="bne")
            nc.vector.tensor_reduce(ex2[:], q_col[:], mybir.AxisListType.X,
                                    AL.add)
            nc.vector.tensor_scalar(out=ex2[:], in0=ex2[:],
                                    scalar1=1.0 / (2 * L), scalar2=None,
                                    op0=AL.mult)
            m2 = scs.tile([D, 1], F32, tag="bnm2")
            nc.vector.tensor_mul(m2[:], mean[:], mean[:])
            var = scs.tile([D, 1], F32, tag="bnv")
            nc.vector.tensor_sub(var[:], ex2[:], m2[:])
            mean = mean[:]
            lg = scs.tile([D, 1], F32, tag="bnl")
            nc.scalar.activation(lg[:], var[:], AF.Ln, bias=EPSC[0:D, :])
            isd = scs.tile([D, 1], F32, tag="bni")
            nc.scalar.activation(isd[:], lg[:], AF.Exp, scale=-0.5)
            alpha = scs.tile([D, 1], F32, tag="bna")
            nc.vector.tensor_mul(alpha[:], g_t, isd[:])
            tmid = scs.tile([D, 1], F32, tag="bnt")
            nc.vector.tensor_mul(tmid[:], mean, alpha[:])
            beta = scs.tile([D, 1], F32, tag="bnb")
            nc.vector.tensor_sub(beta[:], b_t, tmid[:])
            # per-batch relu so conv2(b0) can start before b1's is written
            for b in range(2):
                nc.scalar.activation(out_aps[b], ypair[b][:],
                                     AF.Relu, bias=beta[:], scale=alpha[:])

        # conv1 via im2col: one matmul per batch (contract 36)
        x1pad = const.tile([D, 2 * LP], BF16, tag="x1pad", name="x1pad")
        nc.vector.memset(x1pad[:], 0.0)
        y1p = []
        for b in range(2):
            y1b = psP.tile([D, 512], F32, tag="psP", name=f"y1p{b}")
            nc.tensor.matmul(y1b[:], w1c_t,
                             seq36[:, b * 512:(b + 1) * 512],
                             start=True, stop=True, skip_group_check=True)
            y1p.append(y1b)
        x1v = x1pad[:].rearrange("p (b c) -> p b c", b=2)[:, :, 8:520]
        bn_stats_relu(y1p, bn1g_t, bn1b_t, [x1v[:, b, :] for b in range(2)])

        # conv2 -> xr [D, 1024] bf16, then batch-select into xcur
        y2p = []
        for b in range(2):
            y2b = psP.tile([D, 512], F32, tag="psP", name=f"y2p{b}")
            for t in range(9):
                o = 2 * t - 8
                nc.tensor.matmul(
                    y2b[:],
                    w2Tt_t[:, t * D:(t + 1) * D],
                    x1pad[:, b * LP + 8 + o: b * LP + 8 + o + 512],
                    start=(t == 0), stop=(t == 8), skip_group_check=True)
            y2p.append(y2b)
        xr = sc1.tile([D, 1024], BF16, tag="xr")
        bn_stats_relu(y2p, bn2g_t, bn2b_t,
                      [xr[:, 0:512], xr[:, 512:1024]])

        t1 = sc1.tile([D, L], BF16, tag="xsel2")
        nc.gpsimd.tensor_scalar(out=t1[:], in0=xr[:, 512:1024],
                                scalar1=bm1_t, scalar2=None, op0=AL.mult)
        nc.vector.scalar_tensor_tensor(
            out=xcur[0:D, :], in0=xr[:, 0:512], scalar=bm0_t, in1=t1[:],
            op0=AL.mult, op1=AL.add)

        # ---------- transformer layers (single batch) ----------
        def layer_norm(x_in, g_col, b_col, out_name, t2_name):
            # t2 = (x - mean)*rstd; full = g*t2 + b (off the critical path;
            # consumers that can absorb g,b host-side read t2 directly)
            px = pss.tile([E, L], F32, tag="pss")
            nc.tensor.matmul(px[:], PM[:], x_in[:], start=True, stop=True)
            xsq = sc1.tile([E, L], BF16, tag="lnxsq")
            nc.scalar.activation(xsq[:], px[:], AF.Square)
            v1b = sc1.tile([E, L], BF16, tag="lnv1b")
            nc.vector.tensor_copy(v1b[:], px[:])
            pv = pss.tile([E, L], F32, tag="pss")
            nc.tensor.matmul(pv[:], AVG[:], xsq[:], start=True, stop=True)
            lgr = sc1.tile([1, L], F32, tag="invbf", name="lgr")
            nc.scalar.activation(lgr[:], pv[0:1, :], AF.Ln, bias=EPSC[0:1, :])
            rstd = sc1.tile([1, L], BF16, tag="lnr2")
            nc.scalar.activation(rstd[:], lgr[:], AF.Exp, scale=-0.5)
            pb = pss.tile([E, L], F32, tag="pss")
            nc.tensor.matmul(pb[:], ONEROW[0:1, 0:128], rstd[:], start=True,
                             stop=True)
            t2 = vap.tile([E, L], BF16, tag=t2_name)
            nc.vector.tensor_mul(t2[:], v1b[:], pb[:])
            out = vap.tile([E, L], BF16, tag=out_name)
            nc.scalar.activation(out, t2[:], AF.Identity,
                                 bias=b_col, scale=g_col)
            return t2, out

        vT_s = const.tile([128, 1024], BF16, tag="vts", name="vts")
        # per chunk c: [ones(64)|v_h0(64)|ones(64)|v_h1(64)] so the softmax
        # denominator lands at psum partitions 0:64 (recip needs offset 0)
        ones_v = vT_s[:].rearrange("p (c two g) -> p c two g", c=4, two=2)
        nc.vector.memset(ones_v[:, :, :, 0:64], 1.0)

        x = xcur
        xq = xcur   # pre-affine tensor consumed by folded qkv weights
        for l in range(3):
            # --- qkv ---
            pq = pss.tile([E, L], F32, tag="pss")
            nc.tensor.matmul(pq[:], iwTq_t[:, l * E:(l + 1) * E], xq[:],
                             start=True, stop=True)
            q_s = sc1.tile([E, L], BF16, tag="qs")
            nc.scalar.activation(q_s[:], pq[:], AF.Identity,
                                 bias=qb_t[:, l:l + 1])
            pk = pss.tile([E, L], F32, tag="pss")
            nc.tensor.matmul(pk[:], iwTk_t[:, l * E:(l + 1) * E], xq[:],
                             start=True, stop=True)
            k_s = sc1.tile([E, L], BF16, tag="ks")
            nc.vector.tensor_copy(k_s[:], pk[:])
            for c in range(4):
                pv = pss.tile([128, E], F32, tag="pss")
                nc.tensor.matmul(pv[:], xq[:, c * 128:(c + 1) * 128],
                                 iwTv_t[:, l * E:(l + 1) * E],
                                 start=True, stop=True)
                dst = vT_s[:, c * 256:(c + 1) * 256].rearrange(
                    "p (two r) -> p two r", two=2)[:, :, 64:128]
                src = pv[:].rearrange("p (two g) -> p two g", two=2)
                nc.vector.tensor_copy(dst, src)
            # --- attention: per cp emit sc(h0), sc(h1), exp(h0), AV(h0),
            # exp(h1), AV(h1) so the PE never waits on a fresh exp ---
            pos = [pso.tile([128, L], F32, tag="pso", name=f"po{hh}")
                   for hh in range(2)]

            def sc_mm(pp, h, cp):
                for ci in range(2):
                    c = cp * 2 + ci
                    nc.tensor.matmul(
                        pp[:, ci * 512:(ci + 1) * 512],
                        k_s[64 * h:64 * h + 64, c * 128:(c + 1) * 128],
                        q_s[64 * h:64 * h + 64, :],
                        start=True, stop=True, skip_group_check=True)

            def av_mm(at, h, cp):
                for ci in range(2):
                    c = cp * 2 + ci
                    nc.tensor.matmul(
                        pos[h][:], vT_s[:, c * 256 + h * 128:
                                        c * 256 + h * 128 + 128],
                        at[:, ci * 512:(ci + 1) * 512],
                        start=(c == 0), stop=(c == 3),
                        skip_group_check=True)

            for cp in range(2):
                pp0 = psP.tile([128, 1024], F32, tag="psP", name="pp0")
                sc_mm(pp0, 0, cp)
                pp1 = psP.tile([128, 1024], F32, tag="psP", name="pp1")
                sc_mm(pp1, 1, cp)
                at0 = hpool.tile([128, 1024], BF16, tag="at", name="at0")
                nc.scalar.activation(at0[:], pp0[:], AF.Exp, scale=0.125)
                av_mm(at0, 0, cp)
                at1 = hpool.tile([128, 1024], BF16, tag="at", name="at1")
                nc.scalar.activation(at1[:], pp1[:], AF.Exp, scale=0.125)
                av_mm(at1, 1, cp)
            ons = []
            for h in range(2):
                po = pos[h]
                inv_f = sc1.tile([64, L], F32, tag="invbf", name="invf")
                nc.vector.reciprocal_approx_fast(inv_f[:], po[0:64, :])
                on_s = sc1.tile([64, L], BF16, tag=f"ons{h}")
                nc.vector.tensor_mul(on_s[:], po[64:128, :], inv_f[:])
                ons.append(on_s)
            pproj = pso.tile([E, L], F32, tag="pso")
            for h in range(2):
                nc.tensor.matmul(
                    pproj[:], owT_t[:, (l * 2 + h) * E:(l * 2 + h + 1) * E],
                    ons[h][:], start=(h == 0), stop=(h == 1),
                    skip_group_check=True)
            x1 = sc1.tile([E, L], BF16, tag="x1")
            nc.vector.scalar_tensor_tensor(
                out=x1[:], in0=pproj[:], scalar=obe_t[:, l:l + 1], in1=x[:],
                op0=AL.add, op1=AL.add)
            t2f, x1ln = layer_norm(x1, ln1g_t[:, l:l + 1], ln1b_t[:, l:l + 1],
                                   "x1ln", "t2f")
            # --- FFN (paired psum tiles, 2 chunks per 2-bank tile) ---
            pf2 = pso.tile([E, L], F32, tag="pso")

            def ffn_w2(cp, h_bf):
                for ci in range(2):
                    c = cp * 2 + ci
                    nc.tensor.matmul(
                        pf2[:], w2Tf_t[:, (l * 16 + c) * E:
                                       (l * 16 + c + 1) * E],
                        h_bf[:, ci * 512:(ci + 1) * 512],
                        start=(c == 0), stop=(c == 15),
                        skip_group_check=True)

            prev = None
            for cp in range(8):
                pp = psP.tile([E, 1024], F32, tag="psP")
                for ci in range(2):
                    c = cp * 2 + ci
                    nc.tensor.matmul(
                        pp[:, ci * 512:(ci + 1) * 512],
                        w1Tf_t[:, l * FF + c * E: l * FF + (c + 1) * E],
                        t2f[:], start=True, stop=True, skip_group_check=True)
                if prev is not None:
                    ffn_w2(prev[0], prev[1])
                h_bf = hpool.tile([E, 1024], BF16, tag="hbf")
                nc.scalar.activation(h_bf[:, 0:512], pp[:, 0:512], AF.Relu,
                                     bias=fb1_t[:, l * 16 + 2 * cp:
                                                l * 16 + 2 * cp + 1])
                nc.vector.tensor_scalar(
                    out=h_bf[:, 512:1024], in0=pp[:, 512:1024],
                    scalar1=fb1_t[:, l * 16 + 2 * cp + 1: l * 16 + 2 * cp + 2],
                    scalar2=0.0, op0=AL.add, op1=AL.max)
                prev = (cp, h_bf)
            ffn_w2(prev[0], prev[1])
            x2 = sc1.tile([E, L], BF16, tag="x2")
            nc.vector.scalar_tensor_tensor(
                out=x2[:], in0=pf2[:], scalar=fb2_t[:, l:l + 1], in1=x1ln[:],
                op0=AL.add, op1=AL.add)
            xt2, x = layer_norm(x2, ln2g_t[:, l:l + 1], ln2b_t[:, l:l + 1],
                                "xc", "xct2")
            xq = xt2   # folded weights consume the pre-affine tensor

        emb = x
        embq = xq
        nc.sync.dma_start(emb_d.ap(), emb[:])

        # ---------- LC column loop ----------
        # embsel[:, m] = embq[:, q*128+m] via per-core 0/1 quarter masks
        embsel = sc1.tile([128, 128], F32, tag="embsel")
        nc.vector.tensor_scalar(out=embsel[:], in0=embq[:, 0:128],
                                scalar1=packf[:, PF_BMQ:PF_BMQ + 1],
                                scalar2=None, op0=AL.mult)
        for qq in range(1, 4):
            nc.vector.scalar_tensor_tensor(
                out=embsel[:], in0=embq[:, qq * 128:(qq + 1) * 128],
                scalar=packf[:, PF_BMQ + qq:PF_BMQ + qq + 1], in1=embsel[:],
                op0=AL.mult, op1=AL.add)

        # two accumulation groups (rows 0:64, 64:128) so the first half can
        # drain to DRAM while the second half's matmuls still run
        acc = pso.tile([128, L], F32, tag="pso")
        res_t = sc1.tile([128, L], F32, tag="res")
        for m in range(128):
            va = vapool.tile([128, L], BF16, tag="va")
            nc.vector.tensor_scalar(out=va[:], in0=wt_t[:, m * L:(m + 1) * L],
                                    scalar1=embsel[:, m:m + 1], scalar2=None,
                                    op0=AL.mult)
            g0, mm = divmod(m, 64)
            nc.tensor.matmul(acc[64 * g0:64 * g0 + 64, :],
                             ONESW[:, 128 - mm:192 - mm], va[:],
                             start=(mm == 0), stop=(mm == 63),
                             skip_group_check=True)
            if m == 63:
                nc.vector.tensor_copy(res_t[0:64, :], acc[0:64, :])
                nc.sync.dma_start(res_d.ap()[0:64, :], res_t[0:64, :])
        nc.vector.tensor_copy(res_t[64:128, :], acc[64:128, :])
        nc.sync.dma_start(res_d.ap()[64:128, :], res_t[64:128, :])

    nc.compile()
    return nc


def _prep_inputs(inputs):
    f32 = np.float32

    def bf(x):
        return np.ascontiguousarray(x.astype(BF))

    def f(x):
        return np.ascontiguousarray(x.astype(f32))

    seq = inputs["seq"]  # (B, L, 4)
    # padded seq per batch, then im2col over (tap, channel) -> 36 rows
    sp = np.zeros((2, 4, LP), f32)
    for b in range(B):
        sp[b, :, 8:8 + L] = seq[b].T
    seq36 = np.zeros((36, 1024), f32)
    for t in range(9):
        for b in range(B):
            seq36[4 * t:4 * t + 4, b * 512:(b + 1) * 512] = \
                sp[b, :, 2 * t:2 * t + 512]
    w1c = np.zeros((36, D), f32)
    for t in range(9):
        w1c[4 * t:4 * t + 4, :] = inputs["conv1_w"][:, :, t].T

    # ---- f32 pack ----
    packf = np.zeros((128, PF_W), f32)
    packf[0:D, PF_BN1G] = inputs["bn1_g"]
    packf[0:D, PF_BN1B] = inputs["bn1_b"]
    packf[0:D, PF_BN2G] = inputs["bn2_g"]
    packf[0:D, PF_BN2B] = inputs["bn2_b"]

    iw = inputs["attn_in_w"]   # (3, 3E, E)
    ib = inputs["attn_in_b"]   # (3, 3E)
    ow = inputs["attn_out_w"]; ob = inputs["attn_out_b"]
    ln1gA = np.asarray(inputs["ln1_g"], f32); ln1bA = np.asarray(inputs["ln1_b"], f32)
    ln2gA = np.asarray(inputs["ln2_g"], f32); ln2bA = np.asarray(inputs["ln2_b"], f32)
    # layers 1,2 consume the pre-affine LN2 output of the previous layer:
    # fold the previous LN2's gamma into qkv rows, its beta into the biases.
    gin = [np.ones(E, f32), ln2gA[0], ln2gA[1]]
    bin_ = [np.zeros(E, f32), ln2bA[0], ln2bA[1]]
    packf[:, PF_QB:PF_QB + 3] = np.stack(
        [ib[l, 0:E] + iw[l][0:E] @ bin_[l] for l in range(3)], axis=1)
    packf[:, PF_OBE:PF_OBE + 3] = np.stack(
        [ob[l] + ow[l] @ (ib[l, 2 * E:3 * E] + iw[l][2 * E:3 * E] @ bin_[l])
         for l in range(3)], axis=1)
    packf[:, PF_LN1G:PF_LN1G + 3] = inputs["ln1_g"].T
    packf[:, PF_LN1B:PF_LN1B + 3] = inputs["ln1_b"].T
    packf[:, PF_LN2G:PF_LN2G + 3] = inputs["ln2_g"].T
    packf[:, PF_LN2B:PF_LN2B + 3] = inputs["ln2_b"].T
    w1 = inputs["ffn_w1"]; w2 = inputs["ffn_w2"]
    packf[:, PF_FB2:PF_FB2 + 3] = inputs["ffn_b2"].T
    packf[:, PF_FB1:PF_FB1 + 48] = np.concatenate(
        [(inputs["ffn_b1"][l] + w1[l] @ ln1bA[l]).reshape(16, E).T
         for l in range(3)], axis=1)

    # ---- bf16 packs ----
    pack0a = np.zeros((36, P0A_W), f32)
    pack0a[:, P0_SEQ:P0_SEQ + 1024] = seq36
    pack0a[:, P0_W1C:P0_W1C + D] = w1c
    pack0b = np.concatenate(
        [inputs["conv2_w"][:, :, t].T for t in range(9)], axis=1)
    PMh = np.eye(E, dtype=f32) - 1.0 / E   # mean-subtract, symmetric
    packA = np.zeros((128, PA_W), f32)
    packA[:, PA_IWQ:PA_IWQ + 384] = np.concatenate(
        [iw[l][0:E].T * gin[l][:, None] for l in range(3)], axis=1)
    packA[:, PA_IWK:PA_IWK + 384] = np.concatenate(
        [iw[l][E:2 * E].T * gin[l][:, None] for l in range(3)], axis=1)
    packA[:, PA_IWV:PA_IWV + 384] = np.concatenate(
        [iw[l][2 * E:3 * E].T * gin[l][:, None] for l in range(3)], axis=1)
    packA[0:64, PA_OWT:PA_OWT + 768] = np.concatenate(
        [ow[l][:, 64 * h:64 * h + 64].T
         for l in range(3) for h in range(2)], axis=1)
    rv = (np.arange(1, L + 1, dtype=f32) / L)
    packA[0:1, PA_RV:PA_RV + L] = rv[None, :]
    obe_full = [ob[l] + ow[l] @ (ib[l, 2 * E:3 * E] + iw[l][2 * E:3 * E] @ bin_[l])
                for l in range(3)]
    packA[0, PA_OBP:PA_OBP + 384] = np.concatenate(
        [PMh @ obe_full[l] for l in range(3)])
    packA[0, PA_FBP:PA_FBP + 384] = np.concatenate(
        [PMh @ np.asarray(inputs["ffn_b2"][l], f32) for l in range(3)])
    packB = np.zeros((128, PB_W), f32)
    # FFN consumes pre-affine LN1 output: fold LN1 gamma/beta into w1/b1
    packB[:, PB_W1:PB_W1 + 3 * FF] = np.concatenate(
        [w1[l].T * ln1gA[l][:, None] for l in range(3)], axis=1)
    packB[:, PB_W2:PB_W2 + 3 * 16 * E] = np.concatenate(
        [w2[l][:, c * E:(c + 1) * E].T
         for l in range(3) for c in range(16)], axis=1)

    # ---- LC strips; LC consumes pre-affine LN2(layer2) output: fold
    # gamma into the channel coefficients, beta into the host position term
    lc = np.asarray(inputs["lc_w"], dtype=f32)  # (258, L, L)
    A = 0.5 * (lc[0:E] + lc[0:E].transpose(0, 2, 1))
    Bm = 0.5 * (lc[E:2 * E] + lc[E:2 * E].transpose(0, 2, 1))
    g2, b2 = ln2gA[2], ln2bA[2]
    bterm = np.einsum('c,crs->rs', b2, A) + np.einsum('c,crs->rs', b2, Bm)
    A = A * g2[:, None, None]
    Bm = Bm * g2[:, None, None]
    # strip for k: [B[:,k,0:k] | A[:,k:512,k] with diag += B[:,k,k]]
    wts = []
    for q in range(4):
        Wq = np.zeros((128, 128, L), f32)   # (c, m, j)
        for m in range(128):
            k = q * 128 + m
            if k > 0:
                Wq[:, m, 0:k] = Bm[:, k, 0:k]
            Wq[:, m, k:L] = A[:, k:L, k]
            Wq[:, m, k] += Bm[:, k, k]
        wts.append(bf(Wq.reshape(128, 128 * L)))

    # host-side symmetric position term (lower triangle)
    w256, w257 = lc[256], lc[257]
    pos_low = 0.5 * (rv[:, None] * (w256 + w257.T) + rv[None, :] * (w257 + w256.T))
    _cached["pos_low"] = np.tril(pos_low + bterm)

    in_maps = []
    for kcore in range(NCORES):
        g = kcore // 4
        q = kcore % 4
        pfk = packf.copy()
        pfk[0:D, PF_BM0] = 1.0 - g
        pfk[0:D, PF_BM1] = float(g)
        pfk[:, PF_BMQ + q] = 1.0
        m = {
            "packf": pfk,
            "pack0a": bf(pack0a),
            "pack0b": bf(pack0b),
            "packA": bf(packA),
            "packB": bf(packB),
            "wt": wts[q],
        }
        in_maps.append(m)
    return in_maps


def kernel(**inputs):
    if "nc" not in _cached:
        _cached["nc"] = _build()
    nc = _cached["nc"]
    in_maps = _prep_inputs({k: np.asarray(v) for k, v in inputs.items()})
    out = run_bass_kernel_spmd(nc, in_maps, list(range(NCORES)))
    _cached["last"] = out

    pos_low = _cached["pos_low"]
    contact = np.zeros((B, L, L), np.float32)
    for g in range(B):
        P = np.concatenate(
            [out.results[g * 4 + q]["res"] for q in range(4)], axis=0)  # (512, 512) rows=k
        M = np.tril(P, -1) + np.tril(P.T, -1) + np.diag(np.diag(P))
        M += pos_low
        contact[g] = M + M.T - np.diag(np.diag(M))
    return contact.astype(np.float32)
